# revision 36
# baseline (speedup 1.0000x reference)
"""Causal single-head attention on 8 Trainium2 NeuronCores.

Problem: x:[4,2048,1024] f32, W_q/W_k/W_v:[1024,1024] f32.
  q,k,v = x@W; scores = q@k^T/sqrt(d) causal-masked; out = softmax(scores)@v.

Sharding: 8 cores = 4 batches x 2 query-shards (SPMD, identical program,
per-core data). Causal load balance: the 16 query blocks (128 rows each) of a
batch are split between its 2 cores as evens/odds of a pairing chosen so both
cores share one uniform per-pair key-block-count profile [4,8,12,16]
(optimal: 40 key-block iterations/core vs 64 dense).

K/V projections are split between the two cores of a batch (each computes its
sequence-half from its half of x^T) and exchanged with pairwise AllGather
collectives through DRAM bounce buffers — removes the main duplicated
compute. The K gather goes first so attention can start while V gathers
behind the Q^T projection.

Layout trick: everything is computed via out = lhsT.T @ rhs with x fed
PRE-TRANSPOSED from the host (xTh = own half of x[b].T), so the kernel needs
no on-chip transposes at all:
  K^T[d,s] = Wk_blk.T @ xTh    (lhsT = Wk block, natural layout)
  Q^T[d,q] = Wq_blk.T @ xTq
  V[s,d]   = xTh_blk.T @ Wv
  S^T[k,q] = K^T_blk.T @ Q^T   (scores transposed: softmax key-dim = partition)
  P^T      = exp(S^T/32) * mask      (multiplicative post-exp causal mask, host data)
  denom[q] = P^T_blk.T @ ones  ([q,1] per-partition layout for free)
  out[q,d] = P^T_blk.T @ V     (PSUM-accumulated over key blocks)
  out     *= 1/denom           (per-partition broadcast)

All matmul inputs bf16 (1 cycle/row, FWL), f32 PSUM accumulation.
PSUM note: matmul start=True clears has_written for the WHOLE bank, so
interleaved accumulation groups never share a PSUM tile.
"""

import numpy as np
import ml_dtypes

import concourse.mybir as mybir
import concourse.tile as tile
from concourse import bacc
from concourse.bass_utils import run_bass_kernel_spmd
from contextlib import ExitStack

P = 128
S = 2048
D = 1024
H = S // 2  # sequence half per core
NIB = D // P  # 8 contraction blocks
NSB = S // P  # 16 key blocks
NHB = H // P  # 8 key blocks per half
NQB = 8  # local query blocks per core
CNT = [4, 8, 12, 16]  # key blocks per query-block pair (uniform across cores)
G_EVEN = [0, 2, 4, 6, 9, 11, 13, 15]
G_ODD = [1, 3, 5, 7, 8, 10, 12, 14]
BF = mybir.dt.bfloat16
F32 = mybir.dt.float32
SCALE = 1.0 / 32.0  # 1/sqrt(1024)
bf16 = ml_dtypes.bfloat16
PAIRS = [[0, 1], [2, 3], [4, 5], [6, 7]]

_prog_cache = {}


def _build_program(reps: int = 1, parts: str = "all"):
    key = (reps, parts)
    if key in _prog_cache:
        return _prog_cache[key]
    nc = bacc.Bacc("TRN2", target_bir_lowering=False, debug=False, num_devices=8)
    xTh = nc.dram_tensor("xTh", [D, H], BF, kind="ExternalInput").ap()
    xTq = nc.dram_tensor("xTq", [D, NQB * P], BF, kind="ExternalInput").ap()
    Wq = nc.dram_tensor("Wq", [D, D], BF, kind="ExternalInput").ap()
    Wk = nc.dram_tensor("Wk", [D, D], BF, kind="ExternalInput").ap()
    Wv = nc.dram_tensor("Wv", [D, D], BF, kind="ExternalInput").ap()
    masks = nc.dram_tensor("masks", [16, P, 2 * P], BF, kind="ExternalInput").ap()
    O = nc.dram_tensor("O", [NQB * P, D], F32, kind="ExternalOutput").ap()

    with tile.TileContext(nc) as tc:
        for _rep in range(reps):
            _emit_body(nc, tc, xTh, xTq, Wq, Wk, Wv, masks, O, parts)

    nc.compile()
    _prog_cache[key] = nc
    return nc


def _emit_body(nc, tc, xTh, xTq, Wq, Wk, Wv, masks, O, parts="all"):
    with ExitStack() as ctx:
        # Persistent SBUF residents
        res = ctx.enter_context(tc.tile_pool(name="res", bufs=1))
        kT = [res.tile([P, S], BF, tag=f"kT{d}", name=f"kT{d}") for d in range(NIB)]
        qT = [res.tile([P, NQB * P], BF, tag=f"qT{d}", name=f"qT{d}") for d in range(NIB)]
        v = [res.tile([P, D], BF, tag=f"v{s}", name=f"v{s}") for s in range(NSB)]
        ones = res.tile([P, 1], BF, tag="ones", name="ones")
        nc.vector.memset(ones[:], 1.0)

        dram = ctx.enter_context(tc.tile_pool(name="dram", bufs=1, space="DRAM"))
        # one merged bounce: rows 0:NIB*P = K^T half, NIB*P: = V half
        kvin = dram.tile([(NIB + NHB) * P, H], BF, name="kvin")
        if parts != "cc1":
            kout2 = dram.tile([2, NIB * P, H], BF, name="kout2")
            vout2 = dram.tile([2, NHB * P, H], BF, name="vout2")
            kvout = None
        else:
            kvout = dram.tile([2, (NIB + NHB) * P, H], BF, name="kvout")
        kin = kvin[0 : NIB * P, :]
        vin = kvin[NIB * P : (NIB + NHB) * P, :]

        def _kout(e, r0, r1):
            if parts != "cc1":
                return kout2[e, r0:r1, :]
            return kvout[e, r0:r1, :]

        def _vout(e, r0, r1):
            if parts != "cc1":
                return vout2[e, r0:r1, :]
            return kvout[e, NIB * P + r0 : NIB * P + r1, :]

        # ---------------- Phase A: projections ----------------
        with ExitStack() as actx:
            xp = actx.enter_context(tc.tile_pool(name="xp", bufs=1))
            wp = actx.enter_context(tc.tile_pool(name="wp", bufs=1))
            aps = actx.enter_context(tc.tile_pool(name="aps", bufs=2, space="PSUM"))

            xt = [xp.tile([P, H], BF, tag=f"x{i}", name=f"x{i}") for i in range(NIB)]
            xtq = [xp.tile([P, NQB * P], BF, tag=f"xq{i}", name=f"xq{i}") for i in range(NIB)]
            wk = [wp.tile([P, D], BF, tag=f"wk{i}", name=f"wk{i}") for i in range(NIB)]
            wq = [wp.tile([P, D], BF, tag=f"wq{i}", name=f"wq{i}") for i in range(NIB)]
            wv = [wp.tile([P, D], BF, tag=f"wv{i}", name=f"wv{i}") for i in range(NIB)]
            # DMA order matters: kTh needs xt+wk first; wv next (vh), then q
            for i in range(NIB):
                nc.sync.dma_start(xt[i][:], xTh[i * P : (i + 1) * P, :])
                nc.sync.dma_start(wk[i][:], Wk[i * P : (i + 1) * P, :])
            for i in range(NIB):
                nc.sync.dma_start(wv[i][:], Wv[i * P : (i + 1) * P, :])
            for i in range(NIB):
                nc.sync.dma_start(xtq[i][:], xTq[i * P : (i + 1) * P, :])
                nc.sync.dma_start(wq[i][:], Wq[i * P : (i + 1) * P, :])

            def proj2(dst_slices, lhsT, rhs_pairs):
                """dst_slices[n] [128, 512] = sum_i lhsT[i].T @ rhs_pairs[i][n].

                n-innermost so each loaded lhsT is reused by 2 consecutive
                matmuls (halves exposed weight-load time).
                """
                if parts == "mmW":  # timing-only: fixed stationary operand
                    lhsT = [wk[0][:, 0:P] for _ in range(NIB)]
                pss = [
                    aps.tile([P, 512], F32, tag=f"aps{n}", name=f"aps{n}")
                    for n in range(2)
                ]
                for i in range(NIB):
                    for n in range(2):
                        nc.tensor.matmul(
                            pss[n][:], lhsT[i], rhs_pairs[i][n],
                            start=(i == 0), stop=(i == NIB - 1),
                        )
                for n in range(2):
                    nc.vector.tensor_copy(dst_slices[n], pss[n][:])

            _mm_only = parts in ("mmA", "mmW")

            # K^T own half, staged into kT[d][:, 0:H]
            for d in range(NIB):
                proj2(
                    [kT[d][:, n * 512 : (n + 1) * 512] for n in range(2)],
                    [wk[i][:, d * P : (d + 1) * P] for i in range(NIB)],
                    [
                        [xt[i][:, n * 512 : (n + 1) * 512] for n in range(2)]
                        for i in range(NIB)
                    ],
                )
                if not _mm_only:
                    nc.sync.dma_start(kin[d * P : (d + 1) * P, :], kT[d][:, 0:H])
            if parts not in ("cc1", "Anocc", "mmA", "mmW"):
                nc.gpsimd.collective_compute(
                    "AllGather",
                    mybir.AluOpType.bypass,
                    replica_groups=PAIRS,
                    ins=[kin.opt()],
                    outs=[kout2.opt()],
                )

            # V own half, staged into v[0..NHB)
            for s in range(NHB):
                proj2(
                    [v[s][:, n * 512 : (n + 1) * 512] for n in range(2)],
                    [xt[i][:, s * P : (s + 1) * P] for i in range(NIB)],
                    [
                        [wv[i][:, n * 512 : (n + 1) * 512] for n in range(2)]
                        for i in range(NIB)
                    ],
                )
                if not _mm_only:
                    nc.sync.dma_start(vin[s * P : (s + 1) * P, :], v[s][:, 0:D])
            if _mm_only:
                pass
            elif parts == "Anocc":
                for e in range(2):
                    nc.sync.dma_start(kvout[e, :, :], kvin[:, :])
            elif parts == "cc1":
                nc.gpsimd.collective_compute(
                    "AllGather",
                    mybir.AluOpType.bypass,
                    replica_groups=PAIRS,
                    ins=[kvin.opt()],
                    outs=[kvout.opt()],
                )
            else:
                nc.gpsimd.collective_compute(
                    "AllGather",
                    mybir.AluOpType.bypass,
                    replica_groups=PAIRS,
                    ins=[vin.opt()],
                    outs=[vout2.opt()],
                )

            # load back gathered K^T (both halves, true order)
            if not _mm_only:
                for e in range(2):
                    for d in range(NIB):
                        nc.sync.dma_start(
                            kT[d][:, e * H : (e + 1) * H],
                            _kout(e, d * P, (d + 1) * P),
                        )

            # Q^T (overlaps the gathers)
            for d in range(NIB):
                proj2(
                    [qT[d][:, n * 512 : (n + 1) * 512] for n in range(2)],
                    [wq[i][:, d * P : (d + 1) * P] for i in range(NIB)],
                    [
                        [xtq[i][:, n * 512 : (n + 1) * 512] for n in range(2)]
                        for i in range(NIB)
                    ],
                )

            # load back gathered V (true order)
            if not _mm_only:
                for e in range(2):
                    for s in range(NHB):
                        nc.sync.dma_start(
                            v[e * NHB + s][:],
                            _vout(e, s * P, (s + 1) * P),
                        )

        if parts in ("mmA", "mmW"):
            with tc.tile_pool(name="ka", bufs=1) as ka:
                coll = ka.tile([P, 4 * (NIB * 4 + NHB * 2)], F32, name="coll")
                col = 0
                for d in range(NIB):
                    for n in range(2):
                        nc.vector.tensor_copy(
                            coll[:, col : col + 4], kT[d][:, n * 512 : n * 512 + 4]
                        )
                        col += 4
                    for n in range(2):
                        nc.vector.tensor_copy(
                            coll[:, col : col + 4], qT[d][:, n * 512 : n * 512 + 4]
                        )
                        col += 4
                for s in range(NHB):
                    for n in range(2):
                        nc.vector.tensor_copy(
                            coll[:, col : col + 4], v[s][:, n * 512 : n * 512 + 4]
                        )
                        col += 4
                nc.sync.dma_start(O[0:P, 0:col], coll[:, 0:col])
            return

        if parts in ("A", "Anocc"):
            with tc.tile_pool(name="ka", bufs=1) as ka:
                coll = ka.tile([P, 4 * (NIB * 6 + NSB * 2)], F32, name="coll")
                col = 0
                for d in range(NIB):
                    for n in range(4):
                        nc.vector.tensor_copy(
                            coll[:, col : col + 4], kT[d][:, n * 512 : n * 512 + 4]
                        )
                        col += 4
                    for n in range(2):
                        nc.vector.tensor_copy(
                            coll[:, col : col + 4], qT[d][:, n * 512 : n * 512 + 4]
                        )
                        col += 4
                for s in range(NSB):
                    for n in range(2):
                        nc.vector.tensor_copy(
                            coll[:, col : col + 4], v[s][:, n * 512 : n * 512 + 4]
                        )
                        col += 4
                nc.sync.dma_start(O[0:P, 0:col], coll[:, 0:col])
            return

        # ---------------- Phase B: attention ----------------
        mp = ctx.enter_context(tc.tile_pool(name="mp", bufs=1))
        m_tiles = [mp.tile([P, 2 * P], BF, tag=f"m{i}", name=f"m{i}") for i in range(16)]
        for i in range(16):
            nc.sync.dma_start(m_tiles[i][:], masks[i, :, :])

        spool = ctx.enter_context(tc.tile_pool(name="spool", bufs=2, space="PSUM"))
        avpool = ctx.enter_context(tc.tile_pool(name="avpool", bufs=1, space="PSUM"))
        dpool = ctx.enter_context(tc.tile_pool(name="dpool", bufs=1, space="PSUM"))
        pp = ctx.enter_context(tc.tile_pool(name="pp", bufs=3))
        op = ctx.enter_context(tc.tile_pool(name="op", bufs=2))
        rp = ctx.enter_context(tc.tile_pool(name="rp", bufs=2))

        for p in range(4):
            av = [
                [avpool.tile([P, 512], F32, tag=f"av{e}{n}", name=f"av{e}{n}") for n in range(2)]
                for e in range(2)
            ]
            den = [
                dpool.tile([P, 1], F32, tag=f"den{e}", name=f"den{e}")
                for e in range(2)
            ]
            for kb in range(CNT[p]):
                ps_s = spool.tile([P, 2 * P], F32, tag="ps_s", name="ps_s")
                for d in range(NIB):
                    nc.tensor.matmul(
                        ps_s[:],
                        kT[d][:, kb * P : (kb + 1) * P],
                        qT[d][:, p * 2 * P : (p + 1) * 2 * P],
                        start=(d == 0),
                        stop=(d == NIB - 1),
                    )
                pT = pp.tile([P, 2 * P], BF, tag="pT", name="pT")
                nc.scalar.activation(
                    pT[:], ps_s[:], mybir.ActivationFunctionType.Exp, scale=SCALE
                )
                if kb >= CNT[p] - 4:
                    mi = p * 4 + kb - (CNT[p] - 4)
                    pTm = pp.tile([P, 2 * P], BF, tag="pTm", name="pTm")
                    nc.vector.tensor_mul(pTm[:], pT[:], m_tiles[mi][:])
                    pT = pTm
                first, last = (kb == 0), (kb == CNT[p] - 1)
                for e in range(2):
                    lhs = pT[:, e * P : (e + 1) * P]
                    for n in range(2):
                        nc.tensor.matmul(
                            av[e][n][:], lhs, v[kb][:, n * 512 : (n + 1) * 512],
                            start=first, stop=last,
                        )
                    nc.tensor.matmul(
                        den[e][:], lhs, ones[:], start=first, stop=last
                    )
            for e in range(2):
                lj = 2 * p + e
                r = rp.tile([P, 1], F32, tag="r", name="r")
                nc.vector.reciprocal(r[:], den[e][:])
                for n in range(2):
                    osb = op.tile([P, 512], F32, tag="osb", name="osb")
                    nc.vector.tensor_scalar_mul(osb[:], av[e][n][:], r[:])
                    nc.sync.dma_start(
                        O[lj * P : (lj + 1) * P, n * 512 : (n + 1) * 512], osb[:]
                    )


def _build_masks(parity: int) -> np.ndarray:
    """[16, 128, 256] bf16 multiplicative masks in S^T layout [k, q].

    Mask iterations (uniform across cores): the last 4 key blocks of each
    pair. Block value: 1 where key_global <= query_global else 0.
    """
    G = G_EVEN if parity == 0 else G_ODD
    out = np.zeros((16, P, 2 * P), dtype=np.float32)
    tri = (np.arange(P)[:, None] <= np.arange(P)[None, :]).astype(np.float32)
    for p in range(4):
        for j in range(4):
            kb = CNT[p] - 4 + j
            for half in range(2):
                g = G[2 * p + half]
                blk = out[p * 4 + j][:, half * P : (half + 1) * P]
                if kb < g:
                    blk[:] = 1.0
                elif kb == g:
                    blk[:] = tri
                # else stays 0
    return out.astype(bf16)


def kernel(x, W_q, W_k, W_v):
    x = np.asarray(x, dtype=np.float32)
    nc = _build_program()

    Wq16 = np.asarray(W_q, dtype=np.float32).astype(bf16)
    Wk16 = np.asarray(W_k, dtype=np.float32).astype(bf16)
    Wv16 = np.asarray(W_v, dtype=np.float32).astype(bf16)
    masks_by_parity = [_build_masks(0), _build_masks(1)]
    qcols = {}
    for e, G in ((0, G_EVEN), (1, G_ODD)):
        qcols[e] = np.concatenate([np.arange(g * P, (g + 1) * P) for g in G])

    in_maps = []
    for c in range(8):
        b, e = c // 2, c % 2
        xTb = x[b].T.astype(bf16)  # [D, S], contiguous via astype copy
        in_maps.append(
            {
                "xTh": np.ascontiguousarray(xTb[:, e * H : (e + 1) * H]),
                "xTq": np.ascontiguousarray(xTb[:, qcols[e]]),
                "Wq": Wq16,
                "Wk": Wk16,
                "Wv": Wv16,
                "masks": masks_by_parity[e],
            }
        )

    res = run_bass_kernel_spmd(nc, in_maps, core_ids=list(range(8)))

    out = np.empty((x.shape[0], S, D), dtype=np.float32)
    for c in range(8):
        b, e = c // 2, c % 2
        G = G_EVEN if e == 0 else G_ODD
        Oc = res.results[c]["O"]
        for lj, g in enumerate(G):
            out[b, g * P : (g + 1) * P, :] = Oc[lj * P : (lj + 1) * P, :]
    return out


# revision 39
# speedup vs baseline: 1.1242x; 1.1242x over previous
"""Causal single-head attention on 8 Trainium2 NeuronCores.

Problem: x:[4,2048,1024] f32, W_q/W_k/W_v:[1024,1024] f32.
  q,k,v = x@W; scores = q@k^T/sqrt(d) causal-masked; out = softmax(scores)@v.

Sharding: 8 cores = 4 batches x 2 query-shards (SPMD, identical program,
per-core data). Causal load balance: the 16 query blocks (128 rows each) of a
batch are split between its 2 cores as evens/odds of a pairing chosen so both
cores share one uniform per-pair key-block-count profile [4,8,12,16]
(optimal: 40 key-block iterations/core vs 64 dense).

K/V projections are split between the two cores of a batch (each computes its
sequence-half from its half of x^T) and exchanged with pairwise AllGather
collectives through DRAM bounce buffers — removes the main duplicated
compute. The K gather goes first so attention can start while V gathers
behind the Q^T projection.

Layout trick: everything is computed via out = lhsT.T @ rhs with x fed
PRE-TRANSPOSED from the host (xTh = own half of x[b].T), so the kernel needs
no on-chip transposes at all:
  K^T[d,s] = Wk_blk.T @ xTh    (lhsT = Wk block, natural layout)
  Q^T[d,q] = Wq_blk.T @ xTq
  V[s,d]   = xTh_blk.T @ Wv
  S^T[k,q] = K^T_blk.T @ Q^T   (scores transposed: softmax key-dim = partition)
  P^T      = exp(S^T/32) * mask      (multiplicative post-exp causal mask, host data)
  denom[q] = P^T_blk.T @ ones  ([q,1] per-partition layout for free)
  out[q,d] = P^T_blk.T @ V     (PSUM-accumulated over key blocks)
  out     *= 1/denom           (per-partition broadcast)

All matmul inputs bf16 (1 cycle/row, FWL), f32 PSUM accumulation.
PSUM note: matmul start=True clears has_written for the WHOLE bank, so
interleaved accumulation groups never share a PSUM tile.
"""

import numpy as np
import ml_dtypes

import concourse.mybir as mybir
import concourse.tile as tile
from concourse import bacc
from concourse.bass_utils import run_bass_kernel_spmd
from contextlib import ExitStack

P = 128
S = 2048
D = 1024
H = S // 2  # sequence half per core
NIB = D // P  # 8 contraction blocks
NSB = S // P  # 16 key blocks
NHB = H // P  # 8 key blocks per half
NQB = 8  # local query blocks per core
CNT = [4, 8, 12, 16]  # key blocks per query-block pair (uniform across cores)
G_EVEN = [0, 2, 4, 6, 9, 11, 13, 15]
G_ODD = [1, 3, 5, 7, 8, 10, 12, 14]
BF = mybir.dt.bfloat16
F32 = mybir.dt.float32
SCALE = 1.0 / 32.0  # 1/sqrt(1024)
bf16 = ml_dtypes.bfloat16
PAIRS = [[0, 1], [2, 3], [4, 5], [6, 7]]

_prog_cache = {}


def _build_program(reps: int = 1, parts: str = "all"):
    key = (reps, parts)
    if key in _prog_cache:
        return _prog_cache[key]
    nc = bacc.Bacc("TRN2", target_bir_lowering=False, debug=False, num_devices=8)
    xTh = nc.dram_tensor("xTh", [D, H], BF, kind="ExternalInput").ap()
    xTq = nc.dram_tensor("xTq", [D, NQB * P], BF, kind="ExternalInput").ap()
    Wq = nc.dram_tensor("Wq", [D, D], BF, kind="ExternalInput").ap()
    Wk = nc.dram_tensor("Wk", [D, D], BF, kind="ExternalInput").ap()
    Wv = nc.dram_tensor("Wv", [D, D], BF, kind="ExternalInput").ap()
    masks = nc.dram_tensor("masks", [16, P, 2 * P], BF, kind="ExternalInput").ap()
    O = nc.dram_tensor("O", [NQB * P, D], F32, kind="ExternalOutput").ap()

    with tile.TileContext(nc) as tc:
        for _rep in range(reps):
            _emit_body(nc, tc, xTh, xTq, Wq, Wk, Wv, masks, O, parts)

    nc.compile()
    _prog_cache[key] = nc
    return nc


def _emit_body(nc, tc, xTh, xTq, Wq, Wk, Wv, masks, O, parts="all"):
    with ExitStack() as ctx:
        # Persistent SBUF residents
        res = ctx.enter_context(tc.tile_pool(name="res", bufs=1))
        kT = [res.tile([P, S], BF, tag=f"kT{d}", name=f"kT{d}") for d in range(NIB)]
        qT = [res.tile([P, NQB * P], BF, tag=f"qT{d}", name=f"qT{d}") for d in range(NIB)]
        v = [res.tile([P, D], BF, tag=f"v{s}", name=f"v{s}") for s in range(NSB)]
        ones = res.tile([P, 1], BF, tag="ones", name="ones")
        nc.vector.memset(ones[:], 1.0)

        dram = ctx.enter_context(tc.tile_pool(name="dram", bufs=1, space="DRAM"))
        # one merged bounce: rows 0:NIB*P = K^T half, NIB*P: = V half
        kvin = dram.tile([(NIB + NHB) * P, H], BF, name="kvin")
        if parts != "cc1":
            kout2 = dram.tile([2, NIB * P, H], BF, name="kout2")
            vout2 = dram.tile([2, NHB * P, H], BF, name="vout2")
            kvout = None
        else:
            kvout = dram.tile([2, (NIB + NHB) * P, H], BF, name="kvout")
        kin = kvin[0 : NIB * P, :]
        vin = kvin[NIB * P : (NIB + NHB) * P, :]

        def _kout(e, r0, r1):
            if parts != "cc1":
                return kout2[e, r0:r1, :]
            return kvout[e, r0:r1, :]

        def _vout(e, r0, r1):
            if parts != "cc1":
                return vout2[e, r0:r1, :]
            return kvout[e, NIB * P + r0 : NIB * P + r1, :]

        # ---------------- Phase A: projections ----------------
        with ExitStack() as actx:
            xp = actx.enter_context(tc.tile_pool(name="xp", bufs=1))
            wp = actx.enter_context(tc.tile_pool(name="wp", bufs=1))
            aps = actx.enter_context(tc.tile_pool(name="aps", bufs=2, space="PSUM"))

            xt = [xp.tile([P, H], BF, tag=f"x{i}", name=f"x{i}") for i in range(NIB)]
            xtq = [xp.tile([P, NQB * P], BF, tag=f"xq{i}", name=f"xq{i}") for i in range(NIB)]
            wk = [wp.tile([P, D], BF, tag=f"wk{i}", name=f"wk{i}") for i in range(NIB)]
            wq = [wp.tile([P, D], BF, tag=f"wq{i}", name=f"wq{i}") for i in range(NIB)]
            wv = [wp.tile([P, D], BF, tag=f"wv{i}", name=f"wv{i}") for i in range(NIB)]
            # DMA order matters: kTh needs xt+wk first; wv next (vh), then q
            for i in range(NIB):
                nc.sync.dma_start(xt[i][:], xTh[i * P : (i + 1) * P, :])
                nc.sync.dma_start(wk[i][:], Wk[i * P : (i + 1) * P, :])
            for i in range(NIB):
                nc.sync.dma_start(wv[i][:], Wv[i * P : (i + 1) * P, :])
            for i in range(NIB):
                nc.sync.dma_start(xtq[i][:], xTq[i * P : (i + 1) * P, :])
                nc.sync.dma_start(wq[i][:], Wq[i * P : (i + 1) * P, :])

            def proj2(dst_slices, lhsT, rhs_pairs):
                """dst_slices[n] [128, 512] = sum_i lhsT[i].T @ rhs_pairs[i][n].

                n-innermost so each loaded lhsT is reused by 2 consecutive
                matmuls (halves exposed weight-load time).
                """
                if parts == "mmW":  # timing-only: fixed stationary operand
                    lhsT = [wk[0][:, 0:P] for _ in range(NIB)]
                pss = [
                    aps.tile([P, 512], F32, tag=f"aps{n}", name=f"aps{n}")
                    for n in range(2)
                ]
                for i in range(NIB):
                    for n in range(2):
                        nc.tensor.matmul(
                            pss[n][:], lhsT[i], rhs_pairs[i][n],
                            start=(i == 0), stop=(i == NIB - 1),
                        )
                for n in range(2):
                    nc.vector.tensor_copy(dst_slices[n], pss[n][:])

            _mm_only = parts in ("mmA", "mmW")

            # K^T own half, staged into kT[d][:, 0:H]
            for d in range(NIB):
                proj2(
                    [kT[d][:, n * 512 : (n + 1) * 512] for n in range(2)],
                    [wk[i][:, d * P : (d + 1) * P] for i in range(NIB)],
                    [
                        [xt[i][:, n * 512 : (n + 1) * 512] for n in range(2)]
                        for i in range(NIB)
                    ],
                )
                if not _mm_only:
                    nc.sync.dma_start(kin[d * P : (d + 1) * P, :], kT[d][:, 0:H])
            if parts not in ("cc1", "Anocc", "mmA", "mmW"):
                nc.gpsimd.collective_compute(
                    "AllGather",
                    mybir.AluOpType.bypass,
                    replica_groups=PAIRS,
                    ins=[kin.opt()],
                    outs=[kout2.opt()],
                )

            # V own half, staged into v[0..NHB)
            for s in range(NHB):
                proj2(
                    [v[s][:, n * 512 : (n + 1) * 512] for n in range(2)],
                    [xt[i][:, s * P : (s + 1) * P] for i in range(NIB)],
                    [
                        [wv[i][:, n * 512 : (n + 1) * 512] for n in range(2)]
                        for i in range(NIB)
                    ],
                )
                if not _mm_only:
                    nc.sync.dma_start(vin[s * P : (s + 1) * P, :], v[s][:, 0:D])
            if _mm_only:
                pass
            elif parts == "Anocc":
                for e in range(2):
                    nc.sync.dma_start(kvout[e, :, :], kvin[:, :])
            elif parts == "cc1":
                nc.gpsimd.collective_compute(
                    "AllGather",
                    mybir.AluOpType.bypass,
                    replica_groups=PAIRS,
                    ins=[kvin.opt()],
                    outs=[kvout.opt()],
                )
            else:
                nc.gpsimd.collective_compute(
                    "AllGather",
                    mybir.AluOpType.bypass,
                    replica_groups=PAIRS,
                    ins=[vin.opt()],
                    outs=[vout2.opt()],
                )

            # load back gathered K^T (both halves, true order)
            if not _mm_only:
                for e in range(2):
                    for d in range(NIB):
                        nc.sync.dma_start(
                            kT[d][:, e * H : (e + 1) * H],
                            _kout(e, d * P, (d + 1) * P),
                        )

            # Q^T (overlaps the gathers)
            for d in range(NIB):
                proj2(
                    [qT[d][:, n * 512 : (n + 1) * 512] for n in range(2)],
                    [wq[i][:, d * P : (d + 1) * P] for i in range(NIB)],
                    [
                        [xtq[i][:, n * 512 : (n + 1) * 512] for n in range(2)]
                        for i in range(NIB)
                    ],
                )

            # load back gathered V (true order)
            if not _mm_only:
                for e in range(2):
                    for s in range(NHB):
                        nc.sync.dma_start(
                            v[e * NHB + s][:],
                            _vout(e, s * P, (s + 1) * P),
                        )

        if parts in ("mmA", "mmW"):
            with tc.tile_pool(name="ka", bufs=1) as ka:
                coll = ka.tile([P, 4 * (NIB * 4 + NHB * 2)], F32, name="coll")
                col = 0
                for d in range(NIB):
                    for n in range(2):
                        nc.vector.tensor_copy(
                            coll[:, col : col + 4], kT[d][:, n * 512 : n * 512 + 4]
                        )
                        col += 4
                    for n in range(2):
                        nc.vector.tensor_copy(
                            coll[:, col : col + 4], qT[d][:, n * 512 : n * 512 + 4]
                        )
                        col += 4
                for s in range(NHB):
                    for n in range(2):
                        nc.vector.tensor_copy(
                            coll[:, col : col + 4], v[s][:, n * 512 : n * 512 + 4]
                        )
                        col += 4
                nc.sync.dma_start(O[0:P, 0:col], coll[:, 0:col])
            return

        if parts in ("A", "Anocc"):
            with tc.tile_pool(name="ka", bufs=1) as ka:
                coll = ka.tile([P, 4 * (NIB * 6 + NSB * 2)], F32, name="coll")
                col = 0
                for d in range(NIB):
                    for n in range(4):
                        nc.vector.tensor_copy(
                            coll[:, col : col + 4], kT[d][:, n * 512 : n * 512 + 4]
                        )
                        col += 4
                    for n in range(2):
                        nc.vector.tensor_copy(
                            coll[:, col : col + 4], qT[d][:, n * 512 : n * 512 + 4]
                        )
                        col += 4
                for s in range(NSB):
                    for n in range(2):
                        nc.vector.tensor_copy(
                            coll[:, col : col + 4], v[s][:, n * 512 : n * 512 + 4]
                        )
                        col += 4
                nc.sync.dma_start(O[0:P, 0:col], coll[:, 0:col])
            return

        # ---------------- Phase B: attention ----------------
        mp = ctx.enter_context(tc.tile_pool(name="mp", bufs=1))
        m_tiles = [mp.tile([P, 2 * P], BF, tag=f"m{i}", name=f"m{i}") for i in range(16)]
        for i in range(16):
            nc.sync.dma_start(m_tiles[i][:], masks[i, :, :])

        spool = ctx.enter_context(tc.tile_pool(name="spool", bufs=2, space="PSUM"))
        avpool = ctx.enter_context(tc.tile_pool(name="avpool", bufs=1, space="PSUM"))
        dpool = ctx.enter_context(tc.tile_pool(name="dpool", bufs=1, space="PSUM"))
        pp = ctx.enter_context(tc.tile_pool(name="pp", bufs=3))
        op = ctx.enter_context(tc.tile_pool(name="op", bufs=2))
        rp = ctx.enter_context(tc.tile_pool(name="rp", bufs=2))

        for p in range(4):
            av = [
                [avpool.tile([P, 512], F32, tag=f"av{e}{n}", name=f"av{e}{n}") for n in range(2)]
                for e in range(2)
            ]
            den = [
                dpool.tile([P, 1], F32, tag=f"den{e}", name=f"den{e}")
                for e in range(2)
            ]
            for kb in range(CNT[p]):
                ps_s = spool.tile([P, 2 * P], F32, tag="ps_s", name="ps_s")
                for d in range(NIB):
                    nc.tensor.matmul(
                        ps_s[:],
                        kT[d][:, kb * P : (kb + 1) * P],
                        qT[d][:, p * 2 * P : (p + 1) * 2 * P],
                        start=(d == 0),
                        stop=(d == NIB - 1),
                    )
                pT = pp.tile([P, 2 * P], BF, tag="pT", name="pT", bufs=8)
                nc.scalar.activation(
                    pT[:], ps_s[:], mybir.ActivationFunctionType.Exp, scale=SCALE
                )
                if kb >= CNT[p] - 4:
                    mi = p * 4 + kb - (CNT[p] - 4)
                    pTm = pp.tile([P, 2 * P], BF, tag="pTm", name="pTm", bufs=4)
                    nc.vector.tensor_mul(pTm[:], pT[:], m_tiles[mi][:])
                    pT = pTm
                first, last = (kb == 0), (kb == CNT[p] - 1)
                for e in range(2):
                    lhs = pT[:, e * P : (e + 1) * P]
                    for n in range(2):
                        nc.tensor.matmul(
                            av[e][n][:], lhs, v[kb][:, n * 512 : (n + 1) * 512],
                            start=first, stop=last,
                        )
                    nc.tensor.matmul(
                        den[e][:], lhs, ones[:], start=first, stop=last
                    )
            for e in range(2):
                lj = 2 * p + e
                r = rp.tile([P, 1], F32, tag="r", name="r")
                nc.vector.reciprocal(r[:], den[e][:])
                for n in range(2):
                    osb = op.tile([P, 512], F32, tag="osb", name="osb")
                    nc.vector.tensor_scalar_mul(osb[:], av[e][n][:], r[:])
                    nc.sync.dma_start(
                        O[lj * P : (lj + 1) * P, n * 512 : (n + 1) * 512], osb[:]
                    )


def _build_masks(parity: int) -> np.ndarray:
    """[16, 128, 256] bf16 multiplicative masks in S^T layout [k, q].

    Mask iterations (uniform across cores): the last 4 key blocks of each
    pair. Block value: 1 where key_global <= query_global else 0.
    """
    G = G_EVEN if parity == 0 else G_ODD
    out = np.zeros((16, P, 2 * P), dtype=np.float32)
    tri = (np.arange(P)[:, None] <= np.arange(P)[None, :]).astype(np.float32)
    for p in range(4):
        for j in range(4):
            kb = CNT[p] - 4 + j
            for half in range(2):
                g = G[2 * p + half]
                blk = out[p * 4 + j][:, half * P : (half + 1) * P]
                if kb < g:
                    blk[:] = 1.0
                elif kb == g:
                    blk[:] = tri
                # else stays 0
    return out.astype(bf16)


def kernel(x, W_q, W_k, W_v):
    x = np.asarray(x, dtype=np.float32)
    nc = _build_program()

    Wq16 = np.asarray(W_q, dtype=np.float32).astype(bf16)
    Wk16 = np.asarray(W_k, dtype=np.float32).astype(bf16)
    Wv16 = np.asarray(W_v, dtype=np.float32).astype(bf16)
    masks_by_parity = [_build_masks(0), _build_masks(1)]
    qcols = {}
    for e, G in ((0, G_EVEN), (1, G_ODD)):
        qcols[e] = np.concatenate([np.arange(g * P, (g + 1) * P) for g in G])

    in_maps = []
    for c in range(8):
        b, e = c // 2, c % 2
        xTb = x[b].T.astype(bf16)  # [D, S], contiguous via astype copy
        in_maps.append(
            {
                "xTh": np.ascontiguousarray(xTb[:, e * H : (e + 1) * H]),
                "xTq": np.ascontiguousarray(xTb[:, qcols[e]]),
                "Wq": Wq16,
                "Wk": Wk16,
                "Wv": Wv16,
                "masks": masks_by_parity[e],
            }
        )

    # the axon terminal occasionally drops a transient error
    # (UNAVAILABLE / device reset); retry a few times before giving up
    import time as _time

    last_exc = None
    for attempt in range(4):
        try:
            res = run_bass_kernel_spmd(nc, in_maps, core_ids=list(range(8)))
            break
        except Exception as exc:  # noqa: BLE001
            last_exc = exc
            _time.sleep(15 * (attempt + 1))
    else:
        raise last_exc

    out = np.empty((x.shape[0], S, D), dtype=np.float32)
    for c in range(8):
        b, e = c // 2, c % 2
        G = G_EVEN if e == 0 else G_ODD
        Oc = res.results[c]["O"]
        for lj, g in enumerate(G):
            out[b, g * P : (g + 1) * P, :] = Oc[lj * P : (lj + 1) * P, :]
    return out


# revision 43
# speedup vs baseline: 1.3074x; 1.1630x over previous
"""Causal single-head attention on 8 Trainium2 NeuronCores.

Problem: x:[4,2048,1024] f32, W_q/W_k/W_v:[1024,1024] f32.
  q,k,v = x@W; scores = q@k^T/sqrt(d) causal-masked; out = softmax(scores)@v.

Sharding: 8 cores = 4 batches x 2 query-shards (SPMD, identical program,
per-core data). Causal load balance: the 16 query blocks (128 rows each) of a
batch are split between its 2 cores as evens/odds of a pairing chosen so both
cores share one uniform per-pair key-block-count profile [4,8,12,16]
(optimal: 40 key-block iterations/core vs 64 dense).

K/V projections are split between the two cores of a batch (each computes its
sequence-half from its half of x^T) and exchanged with pairwise AllGather
collectives through DRAM bounce buffers — removes the main duplicated
compute. The K gather goes first so attention can start while V gathers
behind the Q^T projection.

Layout trick: everything is computed via out = lhsT.T @ rhs with x fed
PRE-TRANSPOSED from the host (xTh = own half of x[b].T), so the kernel needs
no on-chip transposes at all:
  K^T[d,s] = Wk_blk.T @ xTh    (lhsT = Wk block, natural layout)
  Q^T[d,q] = Wq_blk.T @ xTq
  V[s,d]   = xTh_blk.T @ Wv
  S^T[k,q] = K^T_blk.T @ Q^T   (scores transposed: softmax key-dim = partition)
  P^T      = exp(S^T/32) * mask      (multiplicative post-exp causal mask, host data)
  denom[q] = P^T_blk.T @ ones  ([q,1] per-partition layout for free)
  out[q,d] = P^T_blk.T @ V     (PSUM-accumulated over key blocks)
  out     *= 1/denom           (per-partition broadcast)

All matmul inputs bf16 (1 cycle/row, FWL), f32 PSUM accumulation.
PSUM note: matmul start=True clears has_written for the WHOLE bank, so
interleaved accumulation groups never share a PSUM tile.
"""

import numpy as np
import ml_dtypes

import concourse.mybir as mybir
import concourse.tile as tile
from concourse import bacc
from concourse.bass_utils import run_bass_kernel_spmd
from contextlib import ExitStack

P = 128
S = 2048
D = 1024
H = S // 2  # sequence half per core
NIB = D // P  # 8 contraction blocks
NSB = S // P  # 16 key blocks
NHB = H // P  # 8 key blocks per half
NQB = 8  # local query blocks per core
CNT = [4, 8, 12, 16]  # key blocks per query-block pair (uniform across cores)
G_EVEN = [0, 2, 4, 6, 9, 11, 13, 15]
G_ODD = [1, 3, 5, 7, 8, 10, 12, 14]
BF = mybir.dt.bfloat16
F32 = mybir.dt.float32
SCALE = 1.0 / 32.0  # 1/sqrt(1024)
bf16 = ml_dtypes.bfloat16
PAIRS = [[0, 1], [2, 3], [4, 5], [6, 7]]

_prog_cache = {}


def _build_program(reps: int = 1, parts: str = "all"):
    key = (reps, parts)
    if key in _prog_cache:
        return _prog_cache[key]
    nc = bacc.Bacc("TRN2", target_bir_lowering=False, debug=False, num_devices=8)
    xTh = nc.dram_tensor("xTh", [D, H], BF, kind="ExternalInput").ap()
    xTq = nc.dram_tensor("xTq", [D, NQB * P], BF, kind="ExternalInput").ap()
    Wq = nc.dram_tensor("Wq", [D, D], BF, kind="ExternalInput").ap()
    Wk = nc.dram_tensor("Wk", [D, D], BF, kind="ExternalInput").ap()
    Wv = nc.dram_tensor("Wv", [D, D], BF, kind="ExternalInput").ap()
    masks = nc.dram_tensor("masks", [16, P, 2 * P], BF, kind="ExternalInput").ap()
    O = nc.dram_tensor("O", [NQB * P, D], F32, kind="ExternalOutput").ap()

    with tile.TileContext(nc) as tc:
        for _rep in range(reps):
            _emit_body(nc, tc, xTh, xTq, Wq, Wk, Wv, masks, O, parts)

    nc.compile()
    _prog_cache[key] = nc
    return nc


def _emit_body(nc, tc, xTh, xTq, Wq, Wk, Wv, masks, O, parts="all"):
    with ExitStack() as ctx:
        # Persistent SBUF residents
        res = ctx.enter_context(tc.tile_pool(name="res", bufs=1))
        kT = [res.tile([P, S], BF, tag=f"kT{d}", name=f"kT{d}") for d in range(NIB)]
        qT = [res.tile([P, NQB * P], BF, tag=f"qT{d}", name=f"qT{d}") for d in range(NIB)]
        v = [res.tile([P, D], BF, tag=f"v{s}", name=f"v{s}") for s in range(NSB)]
        ones = res.tile([P, 1], BF, tag="ones", name="ones")
        nc.vector.memset(ones[:], 1.0)

        dram = ctx.enter_context(tc.tile_pool(name="dram", bufs=1, space="DRAM"))
        # one merged bounce: rows 0:NIB*P = K^T half, NIB*P: = V half
        kvin = dram.tile([(NIB + NHB) * P, H], BF, name="kvin")
        if parts == "cc4":
            kouts = [dram.tile([2, NIB * P // 2, H], BF, name=f"kout4_{j}") for j in range(2)]
            vouts = [dram.tile([2, NHB * P // 2, H], BF, name=f"vout4_{j}") for j in range(2)]
            kout2 = vout2 = kvout = None
        elif parts != "cc1":
            kout2 = dram.tile([2, NIB * P, H], BF, name="kout2")
            vout2 = dram.tile([2, NHB * P, H], BF, name="vout2")
            kvout = None
        else:
            kvout = dram.tile([2, (NIB + NHB) * P, H], BF, name="kvout")
        kin = kvin[0 : NIB * P, :]
        vin = kvin[NIB * P : (NIB + NHB) * P, :]

        KH = NIB * P // 2  # rows per k-gather chunk
        VH = NHB * P // 2

        def _kout(e, r0, r1):
            if parts == "cc4":
                j, base = (0, 0) if r1 <= KH else (1, KH)
                return kouts[j][e, r0 - base : r1 - base, :]
            if parts != "cc1":
                return kout2[e, r0:r1, :]
            return kvout[e, r0:r1, :]

        def _vout(e, r0, r1):
            if parts == "cc4":
                j, base = (0, 0) if r1 <= VH else (1, VH)
                return vouts[j][e, r0 - base : r1 - base, :]
            if parts != "cc1":
                return vout2[e, r0:r1, :]
            return kvout[e, NIB * P + r0 : NIB * P + r1, :]

        # ---------------- Phase A: projections ----------------
        with ExitStack() as actx:
            xp = actx.enter_context(tc.tile_pool(name="xp", bufs=1))
            wp = actx.enter_context(tc.tile_pool(name="wp", bufs=1))
            aps = actx.enter_context(tc.tile_pool(name="aps", bufs=2, space="PSUM"))

            xt = [xp.tile([P, H], BF, tag=f"x{i}", name=f"x{i}") for i in range(NIB)]
            xtq = [xp.tile([P, NQB * P], BF, tag=f"xq{i}", name=f"xq{i}") for i in range(NIB)]
            wk = [wp.tile([P, D], BF, tag=f"wk{i}", name=f"wk{i}") for i in range(NIB)]
            wq = [wp.tile([P, D], BF, tag=f"wq{i}", name=f"wq{i}") for i in range(NIB)]
            wv = [wp.tile([P, D], BF, tag=f"wv{i}", name=f"wv{i}") for i in range(NIB)]
            # DMA order matters: kTh needs xt+wk first; wv next (vh), then q
            for i in range(NIB):
                nc.sync.dma_start(xt[i][:], xTh[i * P : (i + 1) * P, :])
                nc.sync.dma_start(wk[i][:], Wk[i * P : (i + 1) * P, :])
            for i in range(NIB):
                nc.sync.dma_start(wv[i][:], Wv[i * P : (i + 1) * P, :])
            for i in range(NIB):
                nc.sync.dma_start(xtq[i][:], xTq[i * P : (i + 1) * P, :])
                nc.sync.dma_start(wq[i][:], Wq[i * P : (i + 1) * P, :])

            def proj2(dst_slices, lhsT, rhs_pairs):
                """dst_slices[n] [128, 512] = sum_i lhsT[i].T @ rhs_pairs[i][n].

                n-innermost so each loaded lhsT is reused by 2 consecutive
                matmuls (halves exposed weight-load time).
                """
                if parts == "mmW":  # timing-only: fixed stationary operand
                    lhsT = [wk[0][:, 0:P] for _ in range(NIB)]
                pss = [
                    aps.tile([P, 512], F32, tag=f"aps{n}", name=f"aps{n}")
                    for n in range(2)
                ]
                for i in range(NIB):
                    for n in range(2):
                        nc.tensor.matmul(
                            pss[n][:], lhsT[i], rhs_pairs[i][n],
                            start=(i == 0), stop=(i == NIB - 1),
                        )
                for n in range(2):
                    nc.vector.tensor_copy(dst_slices[n], pss[n][:])

            _mm_only = parts in ("mmA", "mmW")

            # K^T own half, staged into kT[d][:, 0:H]
            for d in range(NIB):
                proj2(
                    [kT[d][:, n * 512 : (n + 1) * 512] for n in range(2)],
                    [wk[i][:, d * P : (d + 1) * P] for i in range(NIB)],
                    [
                        [xt[i][:, n * 512 : (n + 1) * 512] for n in range(2)]
                        for i in range(NIB)
                    ],
                )
                if not _mm_only:
                    nc.sync.dma_start(kin[d * P : (d + 1) * P, :], kT[d][:, 0:H])
                if parts == "cc4" and d in (NIB // 2 - 1, NIB - 1):
                    j = d // (NIB // 2)
                    nc.gpsimd.collective_compute(
                        "AllGather",
                        mybir.AluOpType.bypass,
                        replica_groups=PAIRS,
                        ins=[kin[j * KH : (j + 1) * KH, :].opt()],
                        outs=[kouts[j].opt()],
                    )
            if parts not in ("cc1", "cc4", "Anocc", "mmA", "mmW"):
                nc.gpsimd.collective_compute(
                    "AllGather",
                    mybir.AluOpType.bypass,
                    replica_groups=PAIRS,
                    ins=[kin.opt()],
                    outs=[kout2.opt()],
                )

            # V own half, staged into v[0..NHB)
            for s in range(NHB):
                proj2(
                    [v[s][:, n * 512 : (n + 1) * 512] for n in range(2)],
                    [xt[i][:, s * P : (s + 1) * P] for i in range(NIB)],
                    [
                        [wv[i][:, n * 512 : (n + 1) * 512] for n in range(2)]
                        for i in range(NIB)
                    ],
                )
                if not _mm_only:
                    nc.sync.dma_start(vin[s * P : (s + 1) * P, :], v[s][:, 0:D])
                if parts == "cc4" and s in (NHB // 2 - 1, NHB - 1):
                    j = s // (NHB // 2)
                    nc.gpsimd.collective_compute(
                        "AllGather",
                        mybir.AluOpType.bypass,
                        replica_groups=PAIRS,
                        ins=[vin[j * VH : (j + 1) * VH, :].opt()],
                        outs=[vouts[j].opt()],
                    )
            if _mm_only or parts == "cc4":
                pass
            elif parts == "Anocc":
                for e in range(2):
                    nc.sync.dma_start(kvout[e, :, :], kvin[:, :])
            elif parts == "cc1":
                nc.gpsimd.collective_compute(
                    "AllGather",
                    mybir.AluOpType.bypass,
                    replica_groups=PAIRS,
                    ins=[kvin.opt()],
                    outs=[kvout.opt()],
                )
            else:
                nc.gpsimd.collective_compute(
                    "AllGather",
                    mybir.AluOpType.bypass,
                    replica_groups=PAIRS,
                    ins=[vin.opt()],
                    outs=[vout2.opt()],
                )

            # load back gathered K^T (both halves, true order)
            if not _mm_only:
                for e in range(2):
                    for d in range(NIB):
                        nc.sync.dma_start(
                            kT[d][:, e * H : (e + 1) * H],
                            _kout(e, d * P, (d + 1) * P),
                        )

            # Q^T (overlaps the gathers)
            for d in range(NIB):
                proj2(
                    [qT[d][:, n * 512 : (n + 1) * 512] for n in range(2)],
                    [wq[i][:, d * P : (d + 1) * P] for i in range(NIB)],
                    [
                        [xtq[i][:, n * 512 : (n + 1) * 512] for n in range(2)]
                        for i in range(NIB)
                    ],
                )

            # load back gathered V (true order)
            if not _mm_only:
                for e in range(2):
                    for s in range(NHB):
                        nc.sync.dma_start(
                            v[e * NHB + s][:],
                            _vout(e, s * P, (s + 1) * P),
                        )

        if parts in ("mmA", "mmW"):
            with tc.tile_pool(name="ka", bufs=1) as ka:
                coll = ka.tile([P, 4 * (NIB * 4 + NHB * 2)], F32, name="coll")
                col = 0
                for d in range(NIB):
                    for n in range(2):
                        nc.vector.tensor_copy(
                            coll[:, col : col + 4], kT[d][:, n * 512 : n * 512 + 4]
                        )
                        col += 4
                    for n in range(2):
                        nc.vector.tensor_copy(
                            coll[:, col : col + 4], qT[d][:, n * 512 : n * 512 + 4]
                        )
                        col += 4
                for s in range(NHB):
                    for n in range(2):
                        nc.vector.tensor_copy(
                            coll[:, col : col + 4], v[s][:, n * 512 : n * 512 + 4]
                        )
                        col += 4
                nc.sync.dma_start(O[0:P, 0:col], coll[:, 0:col])
            return

        if parts in ("A", "Anocc"):
            with tc.tile_pool(name="ka", bufs=1) as ka:
                coll = ka.tile([P, 4 * (NIB * 6 + NSB * 2)], F32, name="coll")
                col = 0
                for d in range(NIB):
                    for n in range(4):
                        nc.vector.tensor_copy(
                            coll[:, col : col + 4], kT[d][:, n * 512 : n * 512 + 4]
                        )
                        col += 4
                    for n in range(2):
                        nc.vector.tensor_copy(
                            coll[:, col : col + 4], qT[d][:, n * 512 : n * 512 + 4]
                        )
                        col += 4
                for s in range(NSB):
                    for n in range(2):
                        nc.vector.tensor_copy(
                            coll[:, col : col + 4], v[s][:, n * 512 : n * 512 + 4]
                        )
                        col += 4
                nc.sync.dma_start(O[0:P, 0:col], coll[:, 0:col])
            return

        # ---------------- Phase B: attention ----------------
        mp = ctx.enter_context(tc.tile_pool(name="mp", bufs=1))
        m_tiles = [mp.tile([P, 2 * P], BF, tag=f"m{i}", name=f"m{i}") for i in range(16)]
        for i in range(16):
            nc.sync.dma_start(m_tiles[i][:], masks[i, :, :])

        spool = ctx.enter_context(tc.tile_pool(name="spool", bufs=2, space="PSUM"))
        avpool = ctx.enter_context(tc.tile_pool(name="avpool", bufs=1, space="PSUM"))
        dpool = ctx.enter_context(tc.tile_pool(name="dpool", bufs=1, space="PSUM"))
        pp = ctx.enter_context(tc.tile_pool(name="pp", bufs=3))
        op = ctx.enter_context(tc.tile_pool(name="op", bufs=2))
        rp = ctx.enter_context(tc.tile_pool(name="rp", bufs=2))

        for p in range(4):
            av = [
                [avpool.tile([P, 512], F32, tag=f"av{e}{n}", name=f"av{e}{n}") for n in range(2)]
                for e in range(2)
            ]
            den = [
                dpool.tile([P, 1], F32, tag=f"den{e}", name=f"den{e}")
                for e in range(2)
            ]
            for kb in range(CNT[p]):
                ps_s = spool.tile([P, 2 * P], F32, tag="ps_s", name="ps_s")
                for d in range(NIB):
                    nc.tensor.matmul(
                        ps_s[:],
                        kT[d][:, kb * P : (kb + 1) * P],
                        qT[d][:, p * 2 * P : (p + 1) * 2 * P],
                        start=(d == 0),
                        stop=(d == NIB - 1),
                    )
                pT = pp.tile([P, 2 * P], BF, tag="pT", name="pT", bufs=8)
                nc.scalar.activation(
                    pT[:], ps_s[:], mybir.ActivationFunctionType.Exp, scale=SCALE
                )
                if kb >= CNT[p] - 4:
                    mi = p * 4 + kb - (CNT[p] - 4)
                    pTm = pp.tile([P, 2 * P], BF, tag="pTm", name="pTm", bufs=4)
                    nc.vector.tensor_mul(pTm[:], pT[:], m_tiles[mi][:])
                    pT = pTm
                first, last = (kb == 0), (kb == CNT[p] - 1)
                for e in range(2):
                    lhs = pT[:, e * P : (e + 1) * P]
                    for n in range(2):
                        nc.tensor.matmul(
                            av[e][n][:], lhs, v[kb][:, n * 512 : (n + 1) * 512],
                            start=first, stop=last,
                        )
                    nc.tensor.matmul(
                        den[e][:], lhs, ones[:], start=first, stop=last
                    )
            for e in range(2):
                lj = 2 * p + e
                r = rp.tile([P, 1], F32, tag="r", name="r")
                nc.vector.reciprocal(r[:], den[e][:])
                for n in range(2):
                    osb = op.tile([P, 512], F32, tag="osb", name="osb")
                    nc.vector.tensor_scalar_mul(osb[:], av[e][n][:], r[:])
                    nc.sync.dma_start(
                        O[lj * P : (lj + 1) * P, n * 512 : (n + 1) * 512], osb[:]
                    )


def _build_masks(parity: int) -> np.ndarray:
    """[16, 128, 256] bf16 multiplicative masks in S^T layout [k, q].

    Mask iterations (uniform across cores): the last 4 key blocks of each
    pair. Block value: 1 where key_global <= query_global else 0.
    """
    G = G_EVEN if parity == 0 else G_ODD
    out = np.zeros((16, P, 2 * P), dtype=np.float32)
    tri = (np.arange(P)[:, None] <= np.arange(P)[None, :]).astype(np.float32)
    for p in range(4):
        for j in range(4):
            kb = CNT[p] - 4 + j
            for half in range(2):
                g = G[2 * p + half]
                blk = out[p * 4 + j][:, half * P : (half + 1) * P]
                if kb < g:
                    blk[:] = 1.0
                elif kb == g:
                    blk[:] = tri
                # else stays 0
    return out.astype(bf16)


def kernel(x, W_q, W_k, W_v):
    x = np.asarray(x, dtype=np.float32)
    nc = _build_program()

    Wq16 = np.asarray(W_q, dtype=np.float32).astype(bf16)
    Wk16 = np.asarray(W_k, dtype=np.float32).astype(bf16)
    Wv16 = np.asarray(W_v, dtype=np.float32).astype(bf16)
    masks_by_parity = [_build_masks(0), _build_masks(1)]
    qcols = {}
    for e, G in ((0, G_EVEN), (1, G_ODD)):
        qcols[e] = np.concatenate([np.arange(g * P, (g + 1) * P) for g in G])

    in_maps = []
    for c in range(8):
        b, e = c // 2, c % 2
        xTb = x[b].T.astype(bf16)  # [D, S], contiguous via astype copy
        in_maps.append(
            {
                "xTh": np.ascontiguousarray(xTb[:, e * H : (e + 1) * H]),
                "xTq": np.ascontiguousarray(xTb[:, qcols[e]]),
                "Wq": Wq16,
                "Wk": Wk16,
                "Wv": Wv16,
                "masks": masks_by_parity[e],
            }
        )

    # the axon terminal occasionally drops a transient error
    # (UNAVAILABLE / device reset); retry a few times before giving up
    import time as _time

    last_exc = None
    for attempt in range(4):
        try:
            res = run_bass_kernel_spmd(nc, in_maps, core_ids=list(range(8)))
            break
        except Exception as exc:  # noqa: BLE001
            last_exc = exc
            _time.sleep(15 * (attempt + 1))
    else:
        raise last_exc

    out = np.empty((x.shape[0], S, D), dtype=np.float32)
    for c in range(8):
        b, e = c // 2, c % 2
        G = G_EVEN if e == 0 else G_ODD
        Oc = res.results[c]["O"]
        for lj, g in enumerate(G):
            out[b, g * P : (g + 1) * P, :] = Oc[lj * P : (lj + 1) * P, :]
    return out
